# revision 7
# baseline (speedup 1.0000x reference)
"""MoE on 8 TRN2 cores — paired-expert F-split variant.

Experts are paired (largest token count with smallest); each pair of
experts (A, B) is assigned to two cores: core 2p takes the first half of
both experts' FFN dim, core 2p+1 the second half. Both cores process all
of A's and B's tokens over their F-half; the host sums the two partial
outputs. This halves the load-imbalance padding versus one-expert-per-core
and splits the per-core GEMM work nearly evenly.

Scheduling notes:
- xet (tokens) is loaded in a few large 3D-AP DMAs (all 8 k-tiles per
  column chunk in one descriptor set) on the gpsimd queue, so the first
  matmul chain waits on one transfer, not eight.
- wt2 tiles ride the sync queue interleaved with the streamed w1 tiles;
  the w1 pool's buffer recycling throttles them to PE pace, so they don't
  steal HBM bandwidth from the startup-critical loads.
- Stage-1 matmuls run over exact token counts (s1/s2 unpadded); stage 2
  runs on the 128-aligned grid. Pad-column h values are uninitialized
  garbage, but each output row depends only on its own token column, so
  garbage stays in pad rows that the host drops.
- y is stored per 512-wide D-chunk right after the gate multiply, so the
  final DRAM store overlaps the last matmul chains.
"""

import sys

import numpy as np

for _p in ("/opt/trn_rl_repo",):
    if _p not in sys.path:
        sys.path.append(_p)

import ml_dtypes
from contextlib import ExitStack

import concourse.bacc as bacc
import concourse.mybir as mybir
from concourse.tile import TileContext
from concourse.bass_utils import run_bass_kernel_spmd

D = 1024
F = 4096
F2 = F // 2
E = 8
TOP_K = 2
P = 128
DT = D // P    # 8 k-tiles for stage 1
FT = F // P    # 32 f tiles per core (16 per expert half)
FT2 = FT // 2
N_CORES = 8

BF16 = mybir.dt.bfloat16
F32 = mybir.dt.float32
NP_BF16 = ml_dtypes.bfloat16

_nc_cache = {}


def _round_up(v, m):
    return ((v + m - 1) // m) * m


def _chunks(total, size):
    out = []
    o = 0
    while o < total:
        out.append((o, min(size, total - o)))
        o += size
    return out


def build_moe_nc(s1, s2, loop_n=1):
    """SPMD program: two expert-half FFNs over segmented tokens.

    Token columns [0, s1) belong to expert A, [s1, s1+s2) to expert B
    (exact counts, not 128-aligned). f-tiles 0..15 are A's F-half,
    16..31 B's F-half. Output grid is 128-aligned: A rows at [0, s1r),
    B rows at [s1r, s1r+s2r).
    """
    s1r = _round_up(s1, P)
    s2r = _round_up(s2, P)
    cpad = s1 + s2          # xet columns (exact)
    grid = s1r + s2r        # output token grid
    ct_n = grid // P
    # (xet col offset, width, f-tile base, grid row offset)
    segs = [(0, s1, 0, 0), (s1, s2, FT2, s1r)]
    # First chunk is small so the very first matmul chain waits on a
    # 256KB transfer instead of 1MB.
    seg_chunks = [[(0, 128)] + [(o + 128, w) for o, w in _chunks(s1 - 128, 512)],
                  _chunks(s2, 512)]

    nc = bacc.Bacc("TRN2", target_bir_lowering=False, debug=False,
                   num_devices=N_CORES)

    xet = nc.dram_tensor("xet", [P, DT, cpad], BF16, kind="ExternalInput")
    wt1 = nc.dram_tensor("wt1", [FT, P, DT * P], BF16, kind="ExternalInput")
    wt2 = nc.dram_tensor("wt2", [F, D], BF16, kind="ExternalInput")
    b1t = nc.dram_tensor("b1t", [P, FT], F32, kind="ExternalInput")
    gt = nc.dram_tensor("gt", [P, ct_n], F32, kind="ExternalInput")
    yo = nc.dram_tensor("y", [grid, D], F32, kind="ExternalOutput")

    # h storage on the 128-aligned grid: A's 16 f-tiles get s1r columns
    # each, B's get s2r.
    def h_off(f):
        if f < FT2:
            return f * s1r
        return FT2 * s1r + (f - FT2) * s2r

    with TileContext(nc) as tc, ExitStack() as ctx:
        const = ctx.enter_context(tc.tile_pool(name="const", bufs=1))
        b1_sb = const.tile([P, FT], F32, tag="b1")
        nc.gpsimd.dma_start(out=b1_sb[:], in_=b1t[:])
        gt_sb = const.tile([P, ct_n], F32, tag="gt")
        nc.gpsimd.dma_start(out=gt_sb[:], in_=gt[:])

        # One tile + one DMA per (segment, column chunk); each transfer
        # carries all 8 k-tiles for that chunk via a 3D access pattern.
        xpool = ctx.enter_context(tc.tile_pool(name="xet", bufs=1))
        xchunk = [[], []]
        for si, (c_off, c_w, f_base, g_off) in enumerate(segs):
            for ci, (c0, cw) in enumerate(seg_chunks[si]):
                t = xpool.tile([P, DT, cw], BF16, tag=f"xet{si}_{ci}")
                nc.gpsimd.dma_start(
                    out=t[:], in_=xet[:, :, c_off + c0:c_off + c0 + cw])
                xchunk[si].append(t)

        # wt2 tiles are allocated up front (resident) but their loads are
        # issued inside the stage-1 loop on the sync queue.
        w2pool = ctx.enter_context(tc.tile_pool(name="wt2", bufs=1))
        wt2_sb = [
            w2pool.tile([P, D], BF16, tag=f"wt2_{f}", name=f"wt2_{f}")
            for f in range(FT)
        ]

        hpool = ctx.enter_context(tc.tile_pool(name="h", bufs=1))
        h_all = hpool.tile([P, FT2 * (s1r + s2r)], BF16, tag="h")

        w1pool = ctx.enter_context(tc.tile_pool(name="wt1", bufs=3))
        ps1pool = ctx.enter_context(tc.tile_pool(name="ps1", bufs=3, space="PSUM"))
        ps2pool = ctx.enter_context(tc.tile_pool(name="ps2", bufs=3, space="PSUM"))
        ypool = ctx.enter_context(tc.tile_pool(name="ys", bufs=4))

        loop_ctx = (
            tc.For_i(0, loop_n, 1, hint_engines=(mybir.EngineType.PE,))
            if loop_n > 1 else None
        )
        if loop_ctx is not None:
            ctx.enter_context(loop_ctx)

        # Stage 1
        def issue_wt2(f):
            nc.sync.dma_start(out=wt2_sb[f][:], in_=wt2[f * P:(f + 1) * P, :])

        it = 0
        for si, (c_off, c_w, f_base, g_off) in enumerate(segs):
            for fi in range(FT2):
                f = f_base + fi
                w1f = w1pool.tile([P, DT * P], BF16, tag="w1f")
                nc.sync.dma_start(out=w1f[:], in_=wt1[f, :, :])
                # wt2 rides behind w1f on the in-order sync queue, lagged
                # 3 iterations so the startup-critical w1 tiles go first
                # (w1f[it] for it>=3 is throttled by pool recycling, which
                # in turn throttles the wt2 stream to PE pace).
                if it >= 3:
                    issue_wt2(it - 3)
                it += 1
                for ci, (c0, cw) in enumerate(seg_chunks[si]):
                    ps = ps1pool.tile([P, 512], F32, tag="ps1")
                    for dt in range(DT):
                        nc.tensor.matmul(
                            ps[:, :cw],
                            w1f[:, dt * P:(dt + 1) * P],
                            xchunk[si][ci][:, dt, :],
                            start=(dt == 0),
                            stop=(dt == DT - 1),
                        )
                    nc.scalar.activation(
                        h_all[:, h_off(f) + c0:h_off(f) + c0 + cw],
                        ps[:, :cw],
                        mybir.ActivationFunctionType.Gelu,
                        bias=b1_sb[:, f:f + 1],
                        scale=1.0,
                    )
        for f in range(FT - 3, FT):
            issue_wt2(f)

        # Stage 2 (on the 128-aligned grid)
        for si, (c_off, c_w, f_base, g_off) in enumerate(segs):
            n_tiles = _round_up(c_w, P) // P
            for ci in range(n_tiles):
                ct = g_off // P + ci
                for dc in range(D // 512):
                    ps2 = ps2pool.tile([P, 512], F32, tag="ps2")
                    for fi in range(FT2):
                        f = f_base + fi
                        nc.tensor.matmul(
                            ps2[:],
                            h_all[:, h_off(f) + ci * P:h_off(f) + ci * P + P],
                            wt2_sb[f][:, dc * 512:(dc + 1) * 512],
                            start=(fi == 0),
                            stop=(fi == FT2 - 1),
                        )
                    ysc = ypool.tile([P, 512], F32, tag="ys")
                    nc.vector.tensor_scalar_mul(
                        ysc[:], ps2[:], gt_sb[:, ct:ct + 1])
                    # gpsimd queue is idle during stage 2; keeping stores
                    # off the sync queue shortens the exit-barrier chain.
                    nc.gpsimd.dma_start(
                        out=yo[ct * P:(ct + 1) * P, dc * 512:(dc + 1) * 512],
                        in_=ysc[:])

    nc.compile()
    return nc


def _get_nc(s1, s2, loop_n=1):
    key = (s1, s2, loop_n)
    if key not in _nc_cache:
        _nc_cache[key] = build_moe_nc(s1, s2, loop_n)
    return _nc_cache[key]


def _route(xf, Wr):
    logits = xf.astype(np.float64) @ Wr.astype(np.float64).T
    order = np.argsort(-logits, axis=1, kind="stable")
    top_i = order[:, :TOP_K]
    top_l = np.take_along_axis(logits, top_i, axis=1)
    m = top_l.max(axis=1, keepdims=True)
    ex = np.exp(top_l - m)
    gate = (ex / ex.sum(axis=1, keepdims=True)).astype(np.float32)
    return top_i, gate


def _tile_w1(block_bf):
    """[F2, D] bf16 -> [FT2, P, DT*P] so each f-tile DMA is contiguous."""
    return np.ascontiguousarray(
        block_bf.reshape(FT2, P, DT, P).transpose(0, 3, 2, 1)
    ).reshape(FT2, P, DT * P)


def make_in_maps(x, Wr, W1, b1, W2, b2):
    B, S, _ = x.shape
    T = B * S
    xf = np.asarray(x, dtype=np.float32).reshape(T, D)
    top_i, gate = _route(xf, np.asarray(Wr, dtype=np.float32))

    idx_list, gate_list = [], []
    for e in range(E):
        t_idx, k_idx = np.nonzero(top_i == e)
        idx_list.append(t_idx.astype(np.int64))
        gate_list.append(gate[t_idx, k_idx])

    counts = np.array([len(i) for i in idx_list])
    order = np.argsort(-counts, kind="stable")
    pairs = [(int(order[i]), int(order[7 - i])) for i in range(4)]
    s1 = max(max(int(counts[a]), 1) for a, _ in pairs)
    s2 = max(max(int(counts[b]), 1) for _, b in pairs)
    s1r = _round_up(s1, P)
    s2r = _round_up(s2, P)
    cpad = s1 + s2
    grid = s1r + s2r
    ct_n = grid // P

    xfT = np.ascontiguousarray(xf.T).astype(NP_BF16)
    W1bf = np.asarray(W1, dtype=np.float32).astype(NP_BF16)   # [E, F, D]
    W2bf = np.asarray(W2, dtype=np.float32).astype(NP_BF16)   # [E, D, F]
    b1f = np.asarray(b1, dtype=np.float32)

    in_maps = []
    for p, (a, b) in enumerate(pairs):
        xe = np.zeros((D, cpad), dtype=NP_BF16)
        xe[:, :counts[a]] = xfT[:, idx_list[a]]
        xe[:, s1:s1 + counts[b]] = xfT[:, idx_list[b]]
        # [D, cpad] -> [P, DT, cpad]
        xet = np.ascontiguousarray(
            xe.reshape(DT, P, cpad).transpose(1, 0, 2))
        gtv = np.zeros(grid, dtype=np.float32)
        gtv[:counts[a]] = gate_list[a]
        gtv[s1r:s1r + counts[b]] = gate_list[b]
        gt = np.ascontiguousarray(gtv.reshape(ct_n, P).T)
        for h in range(2):
            fsl = slice(h * F2, (h + 1) * F2)
            wt1 = np.concatenate(
                [_tile_w1(W1bf[a][fsl, :]), _tile_w1(W1bf[b][fsl, :])], axis=0)
            wt2 = np.concatenate(
                [np.ascontiguousarray(W2bf[a][:, fsl].T),
                 np.ascontiguousarray(W2bf[b][:, fsl].T)], axis=0)
            b1c = np.concatenate(
                [b1f[a][fsl].reshape(FT2, P).T, b1f[b][fsl].reshape(FT2, P).T],
                axis=1)
            in_maps.append({
                "xet": xet,
                "wt1": wt1,
                "wt2": wt2,
                "b1t": np.ascontiguousarray(b1c),
                "gt": gt,
            })
    meta = dict(pairs=pairs, s1=s1, s2=s2, s1r=s1r, s2r=s2r,
                idx_list=idx_list, top_i=top_i, gate=gate, counts=counts)
    return in_maps, meta


def combine(results, meta, x_shape, b2):
    B, S, _ = x_shape
    T = B * S
    s1r = meta["s1r"]
    counts = meta["counts"]
    idx_list = meta["idx_list"]
    out = np.zeros((T, D), dtype=np.float32)
    for p, (a, b) in enumerate(meta["pairs"]):
        ya = results[2 * p]["y"] + results[2 * p + 1]["y"]
        if counts[a]:
            out[idx_list[a]] += ya[:counts[a]]
        if counts[b]:
            out[idx_list[b]] += ya[s1r:s1r + counts[b]]
    b2 = np.asarray(b2, dtype=np.float32)
    if np.any(b2):
        comb = np.zeros((T, E), dtype=np.float32)
        comb[np.arange(T)[:, None], meta["top_i"]] = meta["gate"]
        out += comb @ b2
    return out.reshape(B, S, D)


def kernel(x, Wr, W1, b1, W2, b2):
    in_maps, meta = make_in_maps(x, Wr, W1, b1, W2, b2)
    nc = _get_nc(meta["s1"], meta["s2"])
    res = run_bass_kernel_spmd(nc, in_maps, list(range(N_CORES)))
    return combine(res.results, meta, x.shape, b2)


# revision 8
# speedup vs baseline: 1.0364x; 1.0364x over previous
"""MoE on 8 TRN2 cores — paired-expert F-split variant.

Experts are paired (largest token count with smallest); each pair of
experts (A, B) is assigned to two cores: core 2p takes the first half of
both experts' FFN dim, core 2p+1 the second half. Both cores process all
of A's and B's tokens over their F-half; the host sums the two partial
outputs. This halves the load-imbalance padding versus one-expert-per-core
and splits the per-core GEMM work nearly evenly.

Scheduling notes:
- xet (tokens) is stored chunk-major in DRAM so each column-chunk load is
  one fully contiguous DMA; the first chunk is small (128 cols) so the
  first matmul chain starts as early as possible.
- wt2 tiles ride the sync queue interleaved with the streamed w1 tiles
  (lagged a few iterations); the w1 pool's buffer recycling throttles
  them to PE pace so they don't steal startup HBM bandwidth.
- xet and wt2 each live in ONE SBUF tile with region-tracked deps; fewer
  resident tiles shortens the serialized exit-barrier semaphore chain on
  the PE queue at kernel end.
- Stage-1 matmuls run over exact token counts (s1/s2 unpadded); stage 2
  runs on the 128-aligned grid. Pad-column h values are uninitialized
  garbage, but each output row depends only on its own token column, so
  garbage stays in pad rows that the host drops.
- y is stored per 512-wide D-chunk right after the gate multiply, so the
  final DRAM store overlaps the last matmul chains.
"""

import sys

import numpy as np

for _p in ("/opt/trn_rl_repo",):
    if _p not in sys.path:
        sys.path.append(_p)

import ml_dtypes
from contextlib import ExitStack

import concourse.bacc as bacc
import concourse.mybir as mybir
from concourse.tile import TileContext
from concourse.bass_utils import run_bass_kernel_spmd

D = 1024
F = 4096
F2 = F // 2
E = 8
TOP_K = 2
P = 128
DT = D // P    # 8 k-tiles for stage 1
FT = F // P    # 32 f tiles per core (16 per expert half)
FT2 = FT // 2
N_CORES = 8

BF16 = mybir.dt.bfloat16
F32 = mybir.dt.float32
NP_BF16 = ml_dtypes.bfloat16

_nc_cache = {}


def _round_up(v, m):
    return ((v + m - 1) // m) * m


def _chunks(total, size):
    out = []
    o = 0
    while o < total:
        out.append((o, min(size, total - o)))
        o += size
    return out


def _seg_chunks(s1, s2):
    """Per-segment (col offset within segment, width) chunk lists."""
    a = [(0, 128)] + [(o + 128, w) for o, w in _chunks(s1 - 128, 512)]
    b = _chunks(s2, 512)
    return [a, b]


def build_moe_nc(s1, s2, loop_n=1):
    """SPMD program: two expert-half FFNs over segmented tokens.

    Token columns [0, s1) belong to expert A, [s1, s1+s2) to expert B
    (exact counts, not 128-aligned). f-tiles 0..15 are A's F-half,
    16..31 B's F-half. Output grid is 128-aligned: A rows at [0, s1r),
    B rows at [s1r, s1r+s2r).
    """
    s1r = _round_up(s1, P)
    s2r = _round_up(s2, P)
    cpad = s1 + s2          # xet columns (exact)
    grid = s1r + s2r        # output token grid
    ct_n = grid // P
    # (xet col offset, width, f-tile base, grid row offset)
    segs = [(0, s1, 0, 0), (s1, s2, FT2, s1r)]
    seg_chunks = _seg_chunks(s1, s2)
    # free-dim offset of each chunk in the chunk-major xet layout
    xoff = [[], []]
    acc = 0
    for si in range(2):
        for (c0, cw) in seg_chunks[si]:
            xoff[si].append(acc)
            acc += DT * cw
    assert acc == DT * cpad

    nc = bacc.Bacc("TRN2", target_bir_lowering=False, debug=False,
                   num_devices=N_CORES)

    xet = nc.dram_tensor("xet", [P, DT * cpad], BF16, kind="ExternalInput")
    wt1 = nc.dram_tensor("wt1", [FT, P, DT * P], BF16, kind="ExternalInput")
    wt2 = nc.dram_tensor("wt2", [F, D], BF16, kind="ExternalInput")
    b1t = nc.dram_tensor("b1t", [P, FT], F32, kind="ExternalInput")
    gt = nc.dram_tensor("gt", [P, ct_n], F32, kind="ExternalInput")
    yo = nc.dram_tensor("y", [grid, D], F32, kind="ExternalOutput")

    # h storage on the 128-aligned grid: A's 16 f-tiles get s1r columns
    # each, B's get s2r.
    def h_off(f):
        if f < FT2:
            return f * s1r
        return FT2 * s1r + (f - FT2) * s2r

    with TileContext(nc) as tc, ExitStack() as ctx:
        const = ctx.enter_context(tc.tile_pool(name="const", bufs=1))
        b1_sb = const.tile([P, FT], F32, tag="b1")
        nc.gpsimd.dma_start(out=b1_sb[:], in_=b1t[:])
        gt_sb = const.tile([P, ct_n], F32, tag="gt")
        nc.gpsimd.dma_start(out=gt_sb[:], in_=gt[:])

        # One contiguous DMA per (segment, column chunk), all into a
        # single chunk-major tile.
        xpool = ctx.enter_context(tc.tile_pool(name="xet", bufs=1))
        xet_sb = xpool.tile([P, DT * cpad], BF16, tag="xet")
        for si in range(2):
            for ci, (c0, cw) in enumerate(seg_chunks[si]):
                o = xoff[si][ci]
                nc.gpsimd.dma_start(
                    out=xet_sb[:, o:o + DT * cw], in_=xet[:, o:o + DT * cw])

        # wt2 lives in one tile; per-f loads are issued inside the
        # stage-1 loop on the sync queue.
        w2pool = ctx.enter_context(tc.tile_pool(name="wt2", bufs=1))
        wt2_sb = w2pool.tile([P, FT * D], BF16, tag="wt2")

        hpool = ctx.enter_context(tc.tile_pool(name="h", bufs=1))
        h_all = hpool.tile([P, FT2 * (s1r + s2r)], BF16, tag="h")

        w1pool = ctx.enter_context(tc.tile_pool(name="wt1", bufs=4))
        ps1pool = ctx.enter_context(tc.tile_pool(name="ps1", bufs=3, space="PSUM"))
        ps2pool = ctx.enter_context(tc.tile_pool(name="ps2", bufs=3, space="PSUM"))
        ypool = ctx.enter_context(tc.tile_pool(name="ys", bufs=4))

        loop_ctx = (
            tc.For_i(0, loop_n, 1, hint_engines=(mybir.EngineType.PE,))
            if loop_n > 1 else None
        )
        if loop_ctx is not None:
            ctx.enter_context(loop_ctx)

        # Stage 1
        def issue_wt2(f):
            nc.sync.dma_start(
                out=wt2_sb[:, f * D:(f + 1) * D],
                in_=wt2[f * P:(f + 1) * P, :])

        it = 0
        for si, (c_off, c_w, f_base, g_off) in enumerate(segs):
            for fi in range(FT2):
                f = f_base + fi
                w1f = w1pool.tile([P, DT * P], BF16, tag="w1f")
                nc.sync.dma_start(out=w1f[:], in_=wt1[f, :, :])
                # wt2 rides behind w1f on the in-order sync queue, lagged
                # 3 iterations so the startup-critical w1 tiles go first.
                if it >= 3:
                    issue_wt2(it - 3)
                it += 1
                for ci, (c0, cw) in enumerate(seg_chunks[si]):
                    o = xoff[si][ci]
                    ps = ps1pool.tile([P, 512], F32, tag="ps1")
                    for dt in range(DT):
                        nc.tensor.matmul(
                            ps[:, :cw],
                            w1f[:, dt * P:(dt + 1) * P],
                            xet_sb[:, o + dt * cw:o + (dt + 1) * cw],
                            start=(dt == 0),
                            stop=(dt == DT - 1),
                        )
                    nc.scalar.activation(
                        h_all[:, h_off(f) + c0:h_off(f) + c0 + cw],
                        ps[:, :cw],
                        mybir.ActivationFunctionType.Gelu,
                        bias=b1_sb[:, f:f + 1],
                        scale=1.0,
                    )
        for f in range(FT - 3, FT):
            issue_wt2(f)

        # Stage 2 (on the 128-aligned grid)
        for si, (c_off, c_w, f_base, g_off) in enumerate(segs):
            n_tiles = _round_up(c_w, P) // P
            for ci in range(n_tiles):
                ct = g_off // P + ci
                for dc in range(D // 512):
                    ps2 = ps2pool.tile([P, 512], F32, tag="ps2")
                    for fi in range(FT2):
                        f = f_base + fi
                        nc.tensor.matmul(
                            ps2[:],
                            h_all[:, h_off(f) + ci * P:h_off(f) + ci * P + P],
                            wt2_sb[:, f * D + dc * 512:f * D + (dc + 1) * 512],
                            start=(fi == 0),
                            stop=(fi == FT2 - 1),
                        )
                    ysc = ypool.tile([P, 512], F32, tag="ys")
                    nc.vector.tensor_scalar_mul(
                        ysc[:], ps2[:], gt_sb[:, ct:ct + 1])
                    nc.gpsimd.dma_start(
                        out=yo[ct * P:(ct + 1) * P, dc * 512:(dc + 1) * 512],
                        in_=ysc[:])

    nc.compile()
    return nc


def _get_nc(s1, s2, loop_n=1):
    key = (s1, s2, loop_n)
    if key not in _nc_cache:
        _nc_cache[key] = build_moe_nc(s1, s2, loop_n)
    return _nc_cache[key]


def _route(xf, Wr):
    logits = xf.astype(np.float64) @ Wr.astype(np.float64).T
    order = np.argsort(-logits, axis=1, kind="stable")
    top_i = order[:, :TOP_K]
    top_l = np.take_along_axis(logits, top_i, axis=1)
    m = top_l.max(axis=1, keepdims=True)
    ex = np.exp(top_l - m)
    gate = (ex / ex.sum(axis=1, keepdims=True)).astype(np.float32)
    return top_i, gate


def _tile_w1(block_bf):
    """[F2, D] bf16 -> [FT2, P, DT*P] so each f-tile DMA is contiguous."""
    return np.ascontiguousarray(
        block_bf.reshape(FT2, P, DT, P).transpose(0, 3, 2, 1)
    ).reshape(FT2, P, DT * P)


def make_in_maps(x, Wr, W1, b1, W2, b2):
    B, S, _ = x.shape
    T = B * S
    xf = np.asarray(x, dtype=np.float32).reshape(T, D)
    top_i, gate = _route(xf, np.asarray(Wr, dtype=np.float32))

    idx_list, gate_list = [], []
    for e in range(E):
        t_idx, k_idx = np.nonzero(top_i == e)
        idx_list.append(t_idx.astype(np.int64))
        gate_list.append(gate[t_idx, k_idx])

    counts = np.array([len(i) for i in idx_list])
    order = np.argsort(-counts, kind="stable")
    pairs = [(int(order[i]), int(order[7 - i])) for i in range(4)]
    s1 = max(max(int(counts[a]), 1) for a, _ in pairs)
    s2 = max(max(int(counts[b]), 1) for _, b in pairs)
    s1r = _round_up(s1, P)
    s2r = _round_up(s2, P)
    cpad = s1 + s2
    grid = s1r + s2r
    ct_n = grid // P
    seg_chunks = _seg_chunks(s1, s2)

    xfT = np.ascontiguousarray(xf.T).astype(NP_BF16)
    W1bf = np.asarray(W1, dtype=np.float32).astype(NP_BF16)   # [E, F, D]
    W2bf = np.asarray(W2, dtype=np.float32).astype(NP_BF16)   # [E, D, F]
    b1f = np.asarray(b1, dtype=np.float32)

    in_maps = []
    for p, (a, b) in enumerate(pairs):
        xe = np.zeros((D, cpad), dtype=NP_BF16)
        xe[:, :counts[a]] = xfT[:, idx_list[a]]
        xe[:, s1:s1 + counts[b]] = xfT[:, idx_list[b]]
        # [D, cpad] -> [P, DT, cpad] -> chunk-major [P, DT*cpad]
        xe3 = xe.reshape(DT, P, cpad).transpose(1, 0, 2)
        blocks = []
        for si, c_base in ((0, 0), (1, s1)):
            for (c0, cw) in seg_chunks[si]:
                a0 = c_base + c0
                blocks.append(np.ascontiguousarray(
                    xe3[:, :, a0:a0 + cw]).reshape(P, DT * cw))
        xet = np.ascontiguousarray(np.concatenate(blocks, axis=1))
        gtv = np.zeros(grid, dtype=np.float32)
        gtv[:counts[a]] = gate_list[a]
        gtv[s1r:s1r + counts[b]] = gate_list[b]
        gt = np.ascontiguousarray(gtv.reshape(ct_n, P).T)
        for h in range(2):
            fsl = slice(h * F2, (h + 1) * F2)
            wt1 = np.concatenate(
                [_tile_w1(W1bf[a][fsl, :]), _tile_w1(W1bf[b][fsl, :])], axis=0)
            wt2 = np.concatenate(
                [np.ascontiguousarray(W2bf[a][:, fsl].T),
                 np.ascontiguousarray(W2bf[b][:, fsl].T)], axis=0)
            b1c = np.concatenate(
                [b1f[a][fsl].reshape(FT2, P).T, b1f[b][fsl].reshape(FT2, P).T],
                axis=1)
            in_maps.append({
                "xet": xet,
                "wt1": wt1,
                "wt2": wt2,
                "b1t": np.ascontiguousarray(b1c),
                "gt": gt,
            })
    meta = dict(pairs=pairs, s1=s1, s2=s2, s1r=s1r, s2r=s2r,
                idx_list=idx_list, top_i=top_i, gate=gate, counts=counts)
    return in_maps, meta


def combine(results, meta, x_shape, b2):
    B, S, _ = x_shape
    T = B * S
    s1r = meta["s1r"]
    counts = meta["counts"]
    idx_list = meta["idx_list"]
    out = np.zeros((T, D), dtype=np.float32)
    for p, (a, b) in enumerate(meta["pairs"]):
        ya = results[2 * p]["y"] + results[2 * p + 1]["y"]
        if counts[a]:
            out[idx_list[a]] += ya[:counts[a]]
        if counts[b]:
            out[idx_list[b]] += ya[s1r:s1r + counts[b]]
    b2 = np.asarray(b2, dtype=np.float32)
    if np.any(b2):
        comb = np.zeros((T, E), dtype=np.float32)
        comb[np.arange(T)[:, None], meta["top_i"]] = meta["gate"]
        out += comb @ b2
    return out.reshape(B, S, D)


def kernel(x, Wr, W1, b1, W2, b2):
    in_maps, meta = make_in_maps(x, Wr, W1, b1, W2, b2)
    nc = _get_nc(meta["s1"], meta["s2"])
    res = run_bass_kernel_spmd(nc, in_maps, list(range(N_CORES)))
    return combine(res.results, meta, x.shape, b2)


# revision 14
# speedup vs baseline: 1.0448x; 1.0082x over previous
"""MoE on 8 TRN2 cores — paired-expert F-split variant.

Experts are paired (largest token count with smallest); each pair of
experts (A, B) is assigned to two cores: core 2p takes the first half of
both experts' FFN dim, core 2p+1 the second half. Both cores process all
of A's and B's tokens over their F-half; the host sums the two partial
outputs. This halves the load-imbalance padding versus one-expert-per-core
and splits the per-core GEMM work nearly evenly.

Scheduling notes:
- xet (tokens) is stored chunk-major in DRAM so each column-chunk load is
  one fully contiguous DMA; the first chunk is small (128 cols) so the
  first matmul chain starts as early as possible.
- wt2 tiles ride the sync queue interleaved with the streamed w1 tiles
  (lagged a few iterations); the w1 pool's buffer recycling throttles
  them to PE pace so they don't steal startup HBM bandwidth.
- xet and wt2 each live in ONE SBUF tile with region-tracked deps; fewer
  resident tiles shortens the serialized exit-barrier semaphore chain on
  the PE queue at kernel end.
- Stage-1 matmuls run over exact token counts (s1/s2 unpadded); stage 2
  runs on the 128-aligned grid. Pad-column h values are uninitialized
  garbage, but each output row depends only on its own token column, so
  garbage stays in pad rows that the host drops.
- y is stored per 512-wide D-chunk right after the gate multiply, so the
  final DRAM store overlaps the last matmul chains.
"""

import sys

import numpy as np

for _p in ("/opt/trn_rl_repo",):
    if _p not in sys.path:
        sys.path.append(_p)

import ml_dtypes
from contextlib import ExitStack

import concourse.bacc as bacc
import concourse.mybir as mybir
from concourse.tile import TileContext
from concourse.bass_utils import run_bass_kernel_spmd

D = 1024
F = 4096
F2 = F // 2
E = 8
TOP_K = 2
P = 128
DT = D // P    # 8 k-tiles for stage 1
FT = F // P    # 32 f tiles per core (16 per expert half)
FT2 = FT // 2
N_CORES = 8

BF16 = mybir.dt.bfloat16
F32 = mybir.dt.float32
NP_BF16 = ml_dtypes.bfloat16

_nc_cache = {}


def _round_up(v, m):
    return ((v + m - 1) // m) * m


def _chunks(total, size):
    out = []
    o = 0
    while o < total:
        out.append((o, min(size, total - o)))
        o += size
    return out


def _seg_chunks(s1, s2):
    """Per-segment (col offset within segment, width) chunk lists."""
    a = [(0, 128)] + [(o + 128, w) for o, w in _chunks(s1 - 128, 512)]
    b = _chunks(s2, 512)
    return [a, b]


def build_moe_nc(s1, s2, loop_n=1):
    """SPMD program: two expert-half FFNs over segmented tokens.

    Token columns [0, s1) belong to expert A, [s1, s1+s2) to expert B
    (exact counts, not 128-aligned). f-tiles 0..15 are A's F-half,
    16..31 B's F-half. Output grid is 128-aligned: A rows at [0, s1r),
    B rows at [s1r, s1r+s2r).
    """
    s1r = _round_up(s1, P)
    s2r = _round_up(s2, P)
    cpad = s1 + s2          # xet columns (exact)
    grid = s1r + s2r        # output token grid
    ct_n = grid // P
    # (xet col offset, width, f-tile base, grid row offset)
    segs = [(0, s1, 0, 0), (s1, s2, FT2, s1r)]
    seg_chunks = _seg_chunks(s1, s2)
    # free-dim offset of each chunk in the chunk-major xet layout
    xoff = [[], []]
    acc = 0
    for si in range(2):
        for (c0, cw) in seg_chunks[si]:
            xoff[si].append(acc)
            acc += DT * cw
    assert acc == DT * cpad

    nc = bacc.Bacc("TRN2", target_bir_lowering=False, debug=False,
                   num_devices=N_CORES)

    xet = nc.dram_tensor("xet", [P, DT * cpad], BF16, kind="ExternalInput")
    wt1 = nc.dram_tensor("wt1", [FT, P, DT * P], BF16, kind="ExternalInput")
    # quad-major: [quad, partition, 4 f-tiles * D]
    wt2 = nc.dram_tensor("wt2", [FT // 4, P, 4 * D], BF16, kind="ExternalInput")
    b1t = nc.dram_tensor("b1t", [P, FT], F32, kind="ExternalInput")
    gt = nc.dram_tensor("gt", [P, ct_n], F32, kind="ExternalInput")
    yo = nc.dram_tensor("y", [grid, D], F32, kind="ExternalOutput")

    # h storage on the 128-aligned grid: A's 16 f-tiles get s1r columns
    # each, B's get s2r.
    def h_off(f):
        if f < FT2:
            return f * s1r
        return FT2 * s1r + (f - FT2) * s2r

    with TileContext(nc) as tc, ExitStack() as ctx:
        const = ctx.enter_context(tc.tile_pool(name="const", bufs=1))
        b1_sb = const.tile([P, FT], F32, tag="b1")
        gt_sb = const.tile([P, ct_n], F32, tag="gt")

        # gpsimd queue order: A0, A1, b1, A2, gt, B0, B1, then wt2 quads.
        # The sync queue carries only the w1 stream (+ ys stores later),
        # so the startup-critical loads never fight wt2 for bandwidth.
        xpool = ctx.enter_context(tc.tile_pool(name="xet", bufs=1))
        xet_sb = xpool.tile([P, DT * cpad], BF16, tag="xet")

        def xchunk_dma(si, ci):
            o = xoff[si][ci]
            cw = seg_chunks[si][ci][1]
            nc.gpsimd.dma_start(
                out=xet_sb[:, o:o + DT * cw], in_=xet[:, o:o + DT * cw])

        xchunk_dma(0, 0)
        xchunk_dma(0, 1)
        nc.gpsimd.dma_start(out=b1_sb[:], in_=b1t[:])
        for ci in range(2, len(seg_chunks[0])):
            xchunk_dma(0, ci)
        nc.gpsimd.dma_start(out=gt_sb[:], in_=gt[:])
        for ci in range(len(seg_chunks[1])):
            xchunk_dma(1, ci)

        # wt2 lives in one tile, loaded in 4-f-tile quads on the gpsimd
        # queue behind the xet chunks — done well before stage 2 needs it.
        w2pool = ctx.enter_context(tc.tile_pool(name="wt2", bufs=1))
        wt2_sb = w2pool.tile([P, FT * D], BF16, tag="wt2")
        for q in range(FT // 4):
            nc.gpsimd.dma_start(
                out=wt2_sb[:, q * 4 * D:(q + 1) * 4 * D],
                in_=wt2[q, :, :])

        hpool = ctx.enter_context(tc.tile_pool(name="h", bufs=1))
        h_all = hpool.tile([P, FT2 * (s1r + s2r)], BF16, tag="h")

        w1pool = ctx.enter_context(tc.tile_pool(name="wt1", bufs=4))
        ps1pool = ctx.enter_context(tc.tile_pool(name="ps1", bufs=3, space="PSUM"))
        ps2pool = ctx.enter_context(tc.tile_pool(name="ps2", bufs=3, space="PSUM"))
        ypool = ctx.enter_context(tc.tile_pool(name="ys", bufs=4))

        loop_ctx = (
            tc.For_i(0, loop_n, 1, hint_engines=(mybir.EngineType.PE,))
            if loop_n > 1 else None
        )
        if loop_ctx is not None:
            ctx.enter_context(loop_ctx)

        # Stage 1
        for si, (c_off, c_w, f_base, g_off) in enumerate(segs):
            for fi in range(FT2):
                f = f_base + fi
                w1f = w1pool.tile([P, DT * P], BF16, tag="w1f")
                nc.sync.dma_start(out=w1f[:], in_=wt1[f, :, :])
                for ci, (c0, cw) in enumerate(seg_chunks[si]):
                    o = xoff[si][ci]
                    ps = ps1pool.tile([P, 512], F32, tag="ps1")
                    for dt in range(DT):
                        nc.tensor.matmul(
                            ps[:, :cw],
                            w1f[:, dt * P:(dt + 1) * P],
                            xet_sb[:, o + dt * cw:o + (dt + 1) * cw],
                            start=(dt == 0),
                            stop=(dt == DT - 1),
                        )
                    nc.scalar.activation(
                        h_all[:, h_off(f) + c0:h_off(f) + c0 + cw],
                        ps[:, :cw],
                        mybir.ActivationFunctionType.Gelu,
                        bias=b1_sb[:, f:f + 1],
                        scale=1.0,
                    )

        # Stage 2 (on the 128-aligned grid)
        for si, (c_off, c_w, f_base, g_off) in enumerate(segs):
            n_tiles = _round_up(c_w, P) // P
            for ci in range(n_tiles):
                ct = g_off // P + ci
                ys = ypool.tile([P, D], F32, tag="ys")
                for dc in range(D // 512):
                    ps2 = ps2pool.tile([P, 512], F32, tag="ps2")
                    for fi in range(FT2):
                        f = f_base + fi
                        nc.tensor.matmul(
                            ps2[:],
                            h_all[:, h_off(f) + ci * P:h_off(f) + ci * P + P],
                            wt2_sb[:, f * D + dc * 512:f * D + (dc + 1) * 512],
                            start=(fi == 0),
                            stop=(fi == FT2 - 1),
                        )
                    nc.vector.tensor_scalar_mul(
                        ys[:, dc * 512:(dc + 1) * 512], ps2[:],
                        gt_sb[:, ct:ct + 1])
                nc.sync.dma_start(out=yo[ct * P:(ct + 1) * P, :], in_=ys[:])

    nc.compile()
    return nc


def _get_nc(s1, s2, loop_n=1):
    key = (s1, s2, loop_n)
    if key not in _nc_cache:
        _nc_cache[key] = build_moe_nc(s1, s2, loop_n)
    return _nc_cache[key]


def _route(xf, Wr):
    logits = xf.astype(np.float64) @ Wr.astype(np.float64).T
    order = np.argsort(-logits, axis=1, kind="stable")
    top_i = order[:, :TOP_K]
    top_l = np.take_along_axis(logits, top_i, axis=1)
    m = top_l.max(axis=1, keepdims=True)
    ex = np.exp(top_l - m)
    gate = (ex / ex.sum(axis=1, keepdims=True)).astype(np.float32)
    return top_i, gate


def _tile_w1(block_bf):
    """[F2, D] bf16 -> [FT2, P, DT*P] so each f-tile DMA is contiguous."""
    return np.ascontiguousarray(
        block_bf.reshape(FT2, P, DT, P).transpose(0, 3, 2, 1)
    ).reshape(FT2, P, DT * P)


def make_in_maps(x, Wr, W1, b1, W2, b2):
    B, S, _ = x.shape
    T = B * S
    xf = np.asarray(x, dtype=np.float32).reshape(T, D)
    top_i, gate = _route(xf, np.asarray(Wr, dtype=np.float32))

    idx_list, gate_list = [], []
    for e in range(E):
        t_idx, k_idx = np.nonzero(top_i == e)
        idx_list.append(t_idx.astype(np.int64))
        gate_list.append(gate[t_idx, k_idx])

    counts = np.array([len(i) for i in idx_list])
    order = np.argsort(-counts, kind="stable")
    pairs = [(int(order[i]), int(order[7 - i])) for i in range(4)]
    s1 = max(max(int(counts[a]), 1) for a, _ in pairs)
    s2 = max(max(int(counts[b]), 1) for _, b in pairs)
    s1r = _round_up(s1, P)
    s2r = _round_up(s2, P)
    cpad = s1 + s2
    grid = s1r + s2r
    ct_n = grid // P
    seg_chunks = _seg_chunks(s1, s2)

    xfT = np.ascontiguousarray(xf.T).astype(NP_BF16)
    W1bf = np.asarray(W1, dtype=np.float32).astype(NP_BF16)   # [E, F, D]
    W2bf = np.asarray(W2, dtype=np.float32).astype(NP_BF16)   # [E, D, F]
    b1f = np.asarray(b1, dtype=np.float32)

    in_maps = []
    for p, (a, b) in enumerate(pairs):
        xe = np.zeros((D, cpad), dtype=NP_BF16)
        xe[:, :counts[a]] = xfT[:, idx_list[a]]
        xe[:, s1:s1 + counts[b]] = xfT[:, idx_list[b]]
        # [D, cpad] -> [P, DT, cpad] -> chunk-major [P, DT*cpad]
        xe3 = xe.reshape(DT, P, cpad).transpose(1, 0, 2)
        blocks = []
        for si, c_base in ((0, 0), (1, s1)):
            for (c0, cw) in seg_chunks[si]:
                a0 = c_base + c0
                blocks.append(np.ascontiguousarray(
                    xe3[:, :, a0:a0 + cw]).reshape(P, DT * cw))
        xet = np.ascontiguousarray(np.concatenate(blocks, axis=1))
        gtv = np.zeros(grid, dtype=np.float32)
        gtv[:counts[a]] = gate_list[a]
        gtv[s1r:s1r + counts[b]] = gate_list[b]
        gt = np.ascontiguousarray(gtv.reshape(ct_n, P).T)
        for h in range(2):
            fsl = slice(h * F2, (h + 1) * F2)
            wt1 = np.concatenate(
                [_tile_w1(W1bf[a][fsl, :]), _tile_w1(W1bf[b][fsl, :])], axis=0)
            wt2f = np.concatenate(
                [W2bf[a][:, fsl].T, W2bf[b][:, fsl].T], axis=0)  # [F, D]
            # quad-major: [FT//4, P, 4*D]
            wt2 = np.ascontiguousarray(
                wt2f.reshape(FT // 4, 4, P, D).transpose(0, 2, 1, 3)
            ).reshape(FT // 4, P, 4 * D)
            b1c = np.concatenate(
                [b1f[a][fsl].reshape(FT2, P).T, b1f[b][fsl].reshape(FT2, P).T],
                axis=1)
            in_maps.append({
                "xet": xet,
                "wt1": wt1,
                "wt2": wt2,
                "b1t": np.ascontiguousarray(b1c),
                "gt": gt,
            })
    meta = dict(pairs=pairs, s1=s1, s2=s2, s1r=s1r, s2r=s2r,
                idx_list=idx_list, top_i=top_i, gate=gate, counts=counts)
    return in_maps, meta


def combine(results, meta, x_shape, b2):
    B, S, _ = x_shape
    T = B * S
    s1r = meta["s1r"]
    counts = meta["counts"]
    idx_list = meta["idx_list"]
    out = np.zeros((T, D), dtype=np.float32)
    for p, (a, b) in enumerate(meta["pairs"]):
        ya = results[2 * p]["y"] + results[2 * p + 1]["y"]
        if counts[a]:
            out[idx_list[a]] += ya[:counts[a]]
        if counts[b]:
            out[idx_list[b]] += ya[s1r:s1r + counts[b]]
    b2 = np.asarray(b2, dtype=np.float32)
    if np.any(b2):
        comb = np.zeros((T, E), dtype=np.float32)
        comb[np.arange(T)[:, None], meta["top_i"]] = meta["gate"]
        out += comb @ b2
    return out.reshape(B, S, D)


def kernel(x, Wr, W1, b1, W2, b2):
    in_maps, meta = make_in_maps(x, Wr, W1, b1, W2, b2)
    nc = _get_nc(meta["s1"], meta["s2"])
    res = run_bass_kernel_spmd(nc, in_maps, list(range(N_CORES)))
    return combine(res.results, meta, x.shape, b2)


# revision 16
# speedup vs baseline: 1.0523x; 1.0072x over previous
"""MoE on 8 TRN2 cores — paired-expert F-split variant.

Experts are paired (largest token count with smallest); each pair of
experts (A, B) is assigned to two cores: core 2p takes the first half of
both experts' FFN dim, core 2p+1 the second half. Both cores process all
of A's and B's tokens over their F-half; the host sums the two partial
outputs. This halves the load-imbalance padding versus one-expert-per-core
and splits the per-core GEMM work nearly evenly.

Scheduling notes:
- xet (tokens) is stored chunk-major in DRAM so each column-chunk load is
  one fully contiguous DMA; the first chunk is small (128 cols) so the
  first matmul chain starts as early as possible.
- wt2 tiles ride the sync queue interleaved with the streamed w1 tiles
  (lagged a few iterations); the w1 pool's buffer recycling throttles
  them to PE pace so they don't steal startup HBM bandwidth.
- xet and wt2 each live in ONE SBUF tile with region-tracked deps; fewer
  resident tiles shortens the serialized exit-barrier semaphore chain on
  the PE queue at kernel end.
- Stage-1 matmuls run over exact token counts (s1/s2 unpadded); stage 2
  runs on the 128-aligned grid. Pad-column h values are uninitialized
  garbage, but each output row depends only on its own token column, so
  garbage stays in pad rows that the host drops.
- y is stored per 512-wide D-chunk right after the gate multiply, so the
  final DRAM store overlaps the last matmul chains.
"""

import sys

import numpy as np

for _p in ("/opt/trn_rl_repo",):
    if _p not in sys.path:
        sys.path.append(_p)

import ml_dtypes
from contextlib import ExitStack

import concourse.bacc as bacc
import concourse.mybir as mybir
from concourse.tile import TileContext
from concourse.bass_utils import run_bass_kernel_spmd

D = 1024
F = 4096
F2 = F // 2
E = 8
TOP_K = 2
P = 128
DT = D // P    # 8 k-tiles for stage 1
FT = F // P    # 32 f tiles per core (16 per expert half)
FT2 = FT // 2
N_CORES = 8

BF16 = mybir.dt.bfloat16
F32 = mybir.dt.float32
NP_BF16 = ml_dtypes.bfloat16

_nc_cache = {}


def _round_up(v, m):
    return ((v + m - 1) // m) * m


def _chunks(total, size):
    out = []
    o = 0
    while o < total:
        out.append((o, min(size, total - o)))
        o += size
    return out


def _seg_chunks(s1, s2):
    """Per-segment (col offset within segment, width) chunk lists.

    Segment A leads with small chunks so the cold-start pipeline (PE
    waiting on the first token transfers) stalls in small steps.
    """
    a = [(0, 128), (128, 256), (384, 256)]
    a += [(o + 640, w) for o, w in _chunks(s1 - 640, 512)]
    b = _chunks(s2, 512)
    return [a, b]


def build_moe_nc(s1, s2, loop_n=1):
    """SPMD program: two expert-half FFNs over segmented tokens.

    Token columns [0, s1) belong to expert A, [s1, s1+s2) to expert B
    (exact counts, not 128-aligned). f-tiles 0..15 are A's F-half,
    16..31 B's F-half. Output grid is 128-aligned: A rows at [0, s1r),
    B rows at [s1r, s1r+s2r).
    """
    s1r = _round_up(s1, P)
    s2r = _round_up(s2, P)
    cpad = s1 + s2          # xet columns (exact)
    grid = s1r + s2r        # output token grid
    ct_n = grid // P
    # (xet col offset, width, f-tile base, grid row offset)
    segs = [(0, s1, 0, 0), (s1, s2, FT2, s1r)]
    seg_chunks = _seg_chunks(s1, s2)
    # free-dim offset of each chunk in the chunk-major xet layout
    xoff = [[], []]
    acc = 0
    for si in range(2):
        for (c0, cw) in seg_chunks[si]:
            xoff[si].append(acc)
            acc += DT * cw
    assert acc == DT * cpad

    nc = bacc.Bacc("TRN2", target_bir_lowering=False, debug=False,
                   num_devices=N_CORES)

    xet = nc.dram_tensor("xet", [P, DT * cpad], BF16, kind="ExternalInput")
    wt1 = nc.dram_tensor("wt1", [FT, P, DT * P], BF16, kind="ExternalInput")
    # quad-major: [quad, partition, 4 f-tiles * D]
    wt2 = nc.dram_tensor("wt2", [FT // 4, P, 4 * D], BF16, kind="ExternalInput")
    b1t = nc.dram_tensor("b1t", [P, FT], F32, kind="ExternalInput")
    gt = nc.dram_tensor("gt", [P, ct_n], F32, kind="ExternalInput")
    yo = nc.dram_tensor("y", [grid, D], F32, kind="ExternalOutput")

    # h storage on the 128-aligned grid: A's 16 f-tiles get s1r columns
    # each, B's get s2r.
    def h_off(f):
        if f < FT2:
            return f * s1r
        return FT2 * s1r + (f - FT2) * s2r

    with TileContext(nc) as tc, ExitStack() as ctx:
        const = ctx.enter_context(tc.tile_pool(name="const", bufs=1))
        b1_sb = const.tile([P, FT], F32, tag="b1")
        gt_sb = const.tile([P, ct_n], F32, tag="gt")

        # gpsimd queue order: A0, A1, b1, A2, gt, B0, B1, then wt2 quads.
        # The sync queue carries only the w1 stream (+ ys stores later),
        # so the startup-critical loads never fight wt2 for bandwidth.
        xpool = ctx.enter_context(tc.tile_pool(name="xet", bufs=1))
        xet_sb = xpool.tile([P, DT * cpad], BF16, tag="xet")

        def xchunk_dma(si, ci):
            o = xoff[si][ci]
            cw = seg_chunks[si][ci][1]
            nc.gpsimd.dma_start(
                out=xet_sb[:, o:o + DT * cw], in_=xet[:, o:o + DT * cw])

        xchunk_dma(0, 0)
        xchunk_dma(0, 1)
        nc.gpsimd.dma_start(out=b1_sb[:], in_=b1t[:])
        for ci in range(2, len(seg_chunks[0])):
            xchunk_dma(0, ci)
        nc.gpsimd.dma_start(out=gt_sb[:], in_=gt[:])
        for ci in range(len(seg_chunks[1])):
            xchunk_dma(1, ci)

        # wt2 lives in one tile, loaded in 4-f-tile quads on the gpsimd
        # queue behind the xet chunks — done well before stage 2 needs it.
        w2pool = ctx.enter_context(tc.tile_pool(name="wt2", bufs=1))
        wt2_sb = w2pool.tile([P, FT * D], BF16, tag="wt2")
        for q in range(FT // 4):
            nc.gpsimd.dma_start(
                out=wt2_sb[:, q * 4 * D:(q + 1) * 4 * D],
                in_=wt2[q, :, :])

        hpool = ctx.enter_context(tc.tile_pool(name="h", bufs=1))
        h_all = hpool.tile([P, FT2 * (s1r + s2r)], BF16, tag="h")

        w1pool = ctx.enter_context(tc.tile_pool(name="wt1", bufs=4))
        ps1pool = ctx.enter_context(tc.tile_pool(name="ps1", bufs=3, space="PSUM"))
        ps2pool = ctx.enter_context(tc.tile_pool(name="ps2", bufs=3, space="PSUM"))
        ypool = ctx.enter_context(tc.tile_pool(name="ys", bufs=4))

        loop_ctx = (
            tc.For_i(0, loop_n, 1, hint_engines=(mybir.EngineType.PE,))
            if loop_n > 1 else None
        )
        if loop_ctx is not None:
            ctx.enter_context(loop_ctx)

        # Stage 1
        for si, (c_off, c_w, f_base, g_off) in enumerate(segs):
            for fi in range(FT2):
                f = f_base + fi
                w1f = w1pool.tile([P, DT * P], BF16, tag="w1f")
                nc.sync.dma_start(out=w1f[:], in_=wt1[f, :, :])
                for ci, (c0, cw) in enumerate(seg_chunks[si]):
                    o = xoff[si][ci]
                    ps = ps1pool.tile([P, 512], F32, tag="ps1")
                    for dt in range(DT):
                        nc.tensor.matmul(
                            ps[:, :cw],
                            w1f[:, dt * P:(dt + 1) * P],
                            xet_sb[:, o + dt * cw:o + (dt + 1) * cw],
                            start=(dt == 0),
                            stop=(dt == DT - 1),
                        )
                    nc.scalar.activation(
                        h_all[:, h_off(f) + c0:h_off(f) + c0 + cw],
                        ps[:, :cw],
                        mybir.ActivationFunctionType.Gelu,
                        bias=b1_sb[:, f:f + 1],
                        scale=1.0,
                    )

        # Stage 2 (on the 128-aligned grid)
        for si, (c_off, c_w, f_base, g_off) in enumerate(segs):
            n_tiles = _round_up(c_w, P) // P
            for ci in range(n_tiles):
                ct = g_off // P + ci
                for dc in range(D // 512):
                    ps2 = ps2pool.tile([P, 512], F32, tag="ps2")
                    for fi in range(FT2):
                        f = f_base + fi
                        nc.tensor.matmul(
                            ps2[:],
                            h_all[:, h_off(f) + ci * P:h_off(f) + ci * P + P],
                            wt2_sb[:, f * D + dc * 512:f * D + (dc + 1) * 512],
                            start=(fi == 0),
                            stop=(fi == FT2 - 1),
                        )
                    ysc = ypool.tile([P, 512], F32, tag="ys")
                    nc.vector.tensor_scalar_mul(
                        ysc[:], ps2[:], gt_sb[:, ct:ct + 1])
                    nc.sync.dma_start(
                        out=yo[ct * P:(ct + 1) * P, dc * 512:(dc + 1) * 512],
                        in_=ysc[:])

    nc.compile()
    return nc


def _get_nc(s1, s2, loop_n=1):
    key = (s1, s2, loop_n)
    if key not in _nc_cache:
        _nc_cache[key] = build_moe_nc(s1, s2, loop_n)
    return _nc_cache[key]


def _route(xf, Wr):
    logits = xf.astype(np.float64) @ Wr.astype(np.float64).T
    order = np.argsort(-logits, axis=1, kind="stable")
    top_i = order[:, :TOP_K]
    top_l = np.take_along_axis(logits, top_i, axis=1)
    m = top_l.max(axis=1, keepdims=True)
    ex = np.exp(top_l - m)
    gate = (ex / ex.sum(axis=1, keepdims=True)).astype(np.float32)
    return top_i, gate


def _tile_w1(block_bf):
    """[F2, D] bf16 -> [FT2, P, DT*P] so each f-tile DMA is contiguous."""
    return np.ascontiguousarray(
        block_bf.reshape(FT2, P, DT, P).transpose(0, 3, 2, 1)
    ).reshape(FT2, P, DT * P)


def make_in_maps(x, Wr, W1, b1, W2, b2):
    B, S, _ = x.shape
    T = B * S
    xf = np.asarray(x, dtype=np.float32).reshape(T, D)
    top_i, gate = _route(xf, np.asarray(Wr, dtype=np.float32))

    idx_list, gate_list = [], []
    for e in range(E):
        t_idx, k_idx = np.nonzero(top_i == e)
        idx_list.append(t_idx.astype(np.int64))
        gate_list.append(gate[t_idx, k_idx])

    counts = np.array([len(i) for i in idx_list])
    order = np.argsort(-counts, kind="stable")
    pairs = [(int(order[i]), int(order[7 - i])) for i in range(4)]
    s1 = max(max(int(counts[a]), 1) for a, _ in pairs)
    s2 = max(max(int(counts[b]), 1) for _, b in pairs)
    s1r = _round_up(s1, P)
    s2r = _round_up(s2, P)
    cpad = s1 + s2
    grid = s1r + s2r
    ct_n = grid // P
    seg_chunks = _seg_chunks(s1, s2)

    xfT = np.ascontiguousarray(xf.T).astype(NP_BF16)
    W1bf = np.asarray(W1, dtype=np.float32).astype(NP_BF16)   # [E, F, D]
    W2bf = np.asarray(W2, dtype=np.float32).astype(NP_BF16)   # [E, D, F]
    b1f = np.asarray(b1, dtype=np.float32)

    in_maps = []
    for p, (a, b) in enumerate(pairs):
        xe = np.zeros((D, cpad), dtype=NP_BF16)
        xe[:, :counts[a]] = xfT[:, idx_list[a]]
        xe[:, s1:s1 + counts[b]] = xfT[:, idx_list[b]]
        # [D, cpad] -> [P, DT, cpad] -> chunk-major [P, DT*cpad]
        xe3 = xe.reshape(DT, P, cpad).transpose(1, 0, 2)
        blocks = []
        for si, c_base in ((0, 0), (1, s1)):
            for (c0, cw) in seg_chunks[si]:
                a0 = c_base + c0
                blocks.append(np.ascontiguousarray(
                    xe3[:, :, a0:a0 + cw]).reshape(P, DT * cw))
        xet = np.ascontiguousarray(np.concatenate(blocks, axis=1))
        gtv = np.zeros(grid, dtype=np.float32)
        gtv[:counts[a]] = gate_list[a]
        gtv[s1r:s1r + counts[b]] = gate_list[b]
        gt = np.ascontiguousarray(gtv.reshape(ct_n, P).T)
        for h in range(2):
            fsl = slice(h * F2, (h + 1) * F2)
            wt1 = np.concatenate(
                [_tile_w1(W1bf[a][fsl, :]), _tile_w1(W1bf[b][fsl, :])], axis=0)
            wt2f = np.concatenate(
                [W2bf[a][:, fsl].T, W2bf[b][:, fsl].T], axis=0)  # [F, D]
            # quad-major: [FT//4, P, 4*D]
            wt2 = np.ascontiguousarray(
                wt2f.reshape(FT // 4, 4, P, D).transpose(0, 2, 1, 3)
            ).reshape(FT // 4, P, 4 * D)
            b1c = np.concatenate(
                [b1f[a][fsl].reshape(FT2, P).T, b1f[b][fsl].reshape(FT2, P).T],
                axis=1)
            in_maps.append({
                "xet": xet,
                "wt1": wt1,
                "wt2": wt2,
                "b1t": np.ascontiguousarray(b1c),
                "gt": gt,
            })
    meta = dict(pairs=pairs, s1=s1, s2=s2, s1r=s1r, s2r=s2r,
                idx_list=idx_list, top_i=top_i, gate=gate, counts=counts)
    return in_maps, meta


def combine(results, meta, x_shape, b2):
    B, S, _ = x_shape
    T = B * S
    s1r = meta["s1r"]
    counts = meta["counts"]
    idx_list = meta["idx_list"]
    out = np.zeros((T, D), dtype=np.float32)
    for p, (a, b) in enumerate(meta["pairs"]):
        ya = results[2 * p]["y"] + results[2 * p + 1]["y"]
        if counts[a]:
            out[idx_list[a]] += ya[:counts[a]]
        if counts[b]:
            out[idx_list[b]] += ya[s1r:s1r + counts[b]]
    b2 = np.asarray(b2, dtype=np.float32)
    if np.any(b2):
        comb = np.zeros((T, E), dtype=np.float32)
        comb[np.arange(T)[:, None], meta["top_i"]] = meta["gate"]
        out += comb @ b2
    return out.reshape(B, S, D)


def kernel(x, Wr, W1, b1, W2, b2):
    in_maps, meta = make_in_maps(x, Wr, W1, b1, W2, b2)
    nc = _get_nc(meta["s1"], meta["s2"])
    res = run_bass_kernel_spmd(nc, in_maps, list(range(N_CORES)))
    return combine(res.results, meta, x.shape, b2)
